# revision 1
# baseline (speedup 1.0000x reference)
"""ChirpletKANLinear forward on 8 Trainium2 NeuronCores.

Math (per reference):
    base_out[b,o]  = sum_i silu(x[b,i]) * BW[o,i]
    xs             = (x[b,i] - T[o,i]) / S[o,i]
    out[b,o]       = base_out + sum_i cos(2pi*F*xs)*exp(-0.5*xs^2)*CW[o,i]
                     + bias[o]

Algorithm: each chirplet atom h_oi(x) = CW*cos(2pi*F*(x-T)/S)*
exp(-0.5*((x-T)/S)^2) is a fixed smooth function of the scalar x on
x in [-a, a].  Expand it in a shared Fourier-cosine basis
    h_oi(x) ~= sum_{k<K} c[k,o,i] * cos(k * pi * (x+a) / (2a)),
with coefficients c (DCT-II of h_oi, host-precomputed from the weight
tensors only).  The envelope exp(-0.5 xs^2) vanishes at the domain ends,
so the even periodic extension is smooth and K ~= 2a*max(2pi*F/S)/pi + 12
features suffice for ~2e-3 accuracy.  The whole layer then becomes
    out[b,o] = sum_i sum_k c[k,o,i] * cos(k*theta(x[b,i]))     (+ base path
               with feature silu(x) and coefficients BW, + bias)
i.e. ONE dense matmul with contraction (i,k), plus K cheap cosine feature
tiles that depend on x alone - no per-(o,i) transcendentals.

On-device per k (proven int-phase-wrap pattern):
    DVE: mf = int32(round(2^18 * (k*(xc+a)/(4a) + (k+1)/4)))   (fp32 affine)
    DVE: w  = mf << 14                  (wraps phase mod 2^32 = mod 2pi)
    ACT: f_k = Sin(w * 2pi/2^32) -> bf16  = cos(k*theta) (the extra 1/4
         in the (k+1)/4 term turns Sin into Cos)
    PE : psum[oc] += lhsT(c[k, oc-chunk])^T @ f_k   (4 chunks of 128 outs)

Sharding: 8 cores = 4 in_feature groups (128 each) x 2 batch halves
(512 each).  Every core computes a partial (512 out, 512 batch) fp32
product over its 128 in-features; the host sums the 4 partials per batch
half (the unshard step) - bias+DC folded into the g==0 cores on device.
"""

import math

import numpy as np
import ml_dtypes

import concourse.bass as bass
import concourse.bacc as bacc
import concourse.tile as tile
import concourse.mybir as mybir
from concourse.bass_utils import run_bass_kernel_spmd

B, IN, OUT = 1024, 512, 512
NCORES = 8
NG = 4                      # in-feature groups
NH = 2                      # batch halves
IG = IN // NG               # 128 in-features per group
BH = B // NH                # 512 batch per half
NOC = OUT // 128            # 4 output chunks of 128

A = 5.2                     # cosine-series half-range
K = 48                      # features: k=0 -> silu (base path), 1..K-1 cos
NQ = 512                    # DCT quadrature points (host)

F32 = mybir.dt.float32
I32 = mybir.dt.int32
BF16 = mybir.dt.bfloat16
AF = mybir.ActivationFunctionType
ALU = mybir.AluOpType
TWO_PI = 2.0 * math.pi

TRACE = False
LAST_RESULT = None

_nc_cache = None


def _build_nc(loop_r=None, kk=K, aa=A, mode="full", bufs=(2, 2, 4)):
    nc = bacc.Bacc("TRN2", target_bir_lowering=False, debug=False,
                   num_devices=NCORES)

    xT_d = nc.dram_tensor("xT", [IG, BH], F32, kind="ExternalInput")
    cw_d = nc.dram_tensor("cw", [IG, kk, NOC, 128], BF16,
                          kind="ExternalInput")
    bias_d = nc.dram_tensor("biasv", [128, NOC], F32, kind="ExternalInput")
    out_d = nc.dram_tensor("out", [NOC, 128, BH], F32, kind="ExternalOutput")

    with tile.TileContext(nc) as tc:
        with (
            tc.tile_pool(name="singles", bufs=1) as singles,
            tc.tile_pool(name="mfpool", bufs=bufs[0]) as mfpool,
            tc.tile_pool(name="wpool", bufs=bufs[1]) as wpool,
            tc.tile_pool(name="fpool", bufs=bufs[2]) as fpool,
            tc.tile_pool(name="psum", bufs=1,
                         space=bass.MemorySpace.PSUM) as psump,
        ):
            xT_sb = singles.tile([IG, BH], F32)
            nc.sync.dma_start(xT_sb[:], xT_d[:])
            cw_sb = singles.tile([IG, kk, NOC, 128], BF16)
            nc.sync.dma_start(cw_sb[:], cw_d[:])
            bias_sb = singles.tile([128, NOC], F32)
            nc.sync.dma_start(bias_sb[:], bias_d[:])
            # clamp to [-A, A] so the periodic basis never sees out-of-range x
            xc_sb = singles.tile([IG, BH], F32)
            nc.vector.tensor_scalar(xc_sb, xT_sb, -aa, aa, ALU.max, ALU.min)

            nbank = 8 if mode in ("mm8", "full8") else NOC
            psum_acc = psump.tile([128, nbank, BH], F32)
            fpre = None
            if mode in ("mm", "mm2", "mm8"):
                fpre = singles.tile([IG, BH], BF16)
                nc.scalar.activation(fpre, xT_sb, AF.Silu)

            def compute_body():
                if mode == "empty":
                    f0 = fpool.tile([IG, BH], BF16, tag="f")
                    nc.scalar.activation(f0, xT_sb, AF.Silu)
                    nc.tensor.matmul(
                        psum_acc[:, 0, :], cw_sb[:, 0, 0, :], f0,
                        start=True, stop=True, skip_group_check=True)
                    return
                if mode == "mm":
                    for k in range(kk):
                        last = k == kk - 1
                        for oc in range(NOC):
                            nc.tensor.matmul(
                                psum_acc[:, oc, :], cw_sb[:, k, oc, :], fpre,
                                start=(k == 0), stop=last,
                                skip_group_check=True)
                    return
                if mode == "mm2":
                    # same MM count as "mm" but each stationary operand is
                    # used by TWO consecutive MMs (does HW/walrus skip the
                    # second weight load?)
                    for k in range(kk // 2):
                        last = k == kk // 2 - 1
                        for oc in range(NOC):
                            for rep in range(2):
                                nc.tensor.matmul(
                                    psum_acc[:, oc, :], cw_sb[:, k, oc, :],
                                    fpre,
                                    start=(k == 0 and rep == 0), stop=last,
                                    skip_group_check=True)
                    return
                # full / feat
                f0 = fpool.tile([IG, BH], BF16, tag="f")
                nc.scalar.activation(f0, xT_sb, AF.Silu)
                if mode == "full":
                    for oc in range(NOC):
                        nc.tensor.matmul(
                            psum_acc[:, oc, :], cw_sb[:, 0, oc, :], f0,
                            start=True, stop=False, skip_group_check=True)

                for k in range(1, kk):
                    mf = mfpool.tile([IG, BH], I32, tag="mf")
                    nc.vector.tensor_scalar(
                        mf, xc_sb,
                        float(2 ** 18) * k / (4 * aa),
                        float(2 ** 16) * (k + 1),
                        ALU.mult, ALU.add)
                    w = wpool.tile([IG, BH], I32, tag="w")
                    nc.vector.tensor_scalar(
                        w, mf, 14, 0,
                        ALU.arith_shift_left, ALU.arith_shift_right)
                    f = fpool.tile([IG, BH], BF16, tag="f")
                    nc.scalar.activation(f, w, AF.Sin, bias=0.0,
                                         scale=TWO_PI / 2 ** 32)
                    last = k == kk - 1
                    if mode == "full":
                        for oc in range(NOC):
                            nc.tensor.matmul(
                                psum_acc[:, oc, :], cw_sb[:, k, oc, :], f,
                                start=False, stop=last,
                                skip_group_check=True)
                    elif last:
                        # feat mode: consume the last feature minimally
                        nc.tensor.matmul(
                            psum_acc[:, 0, :], cw_sb[:, 0, 0, :], f,
                            start=True, stop=True, skip_group_check=True)

            if loop_r:
                with tc.For_i(0, loop_r, 1,
                              hint_engines=(mybir.EngineType.Activation,
                                            mybir.EngineType.DVE,
                                            mybir.EngineType.PE)):
                    compute_body()
            else:
                compute_body()

            out_sb = singles.tile([128, NOC, BH], F32)
            for oc in range(NOC):
                nc.scalar.activation(out_sb[:, oc, :], psum_acc[:, oc, :],
                                     AF.Identity,
                                     bias=bias_sb[:, oc:oc + 1], scale=1.0)
                nc.sync.dma_start(out_d[oc], out_sb[:, oc, :])

    nc.compile()
    return nc


def _coeffs(inp, kk=K, aa=A):
    """DCT-II cosine coefficients c[k, o, i] of the chirplet atoms."""
    f = np.float32(inp["frequency"])
    s = np.float32(inp["scale"])
    t = np.float32(inp["translation"])
    cwt = np.float32(inp["chirplet_weights"])
    thq = ((np.arange(NQ) + 0.5) * (math.pi / NQ)).astype(np.float32)
    xq = (2 * aa / math.pi) * thq - aa                     # (NQ,)
    basis = np.cos(np.outer(thq, np.arange(kk))).astype(np.float32)
    basis *= 2.0 / NQ
    basis[:, 0] *= 0.5
    c = np.empty((kk, OUT, IN), np.float32)
    for o0 in range(0, OUT, 64):
        o1 = o0 + 64
        u = (xq[None, None, :] - t[o0:o1, :, None]) / s[o0:o1, :, None]
        h = (np.cos(np.float32(TWO_PI) * f[o0:o1, :, None] * u)
             * np.exp(np.float32(-0.5) * u * u) * cwt[o0:o1, :, None])
        c[:, o0:o1, :] = np.einsum("oiq,qk->koi", h, basis, optimize=True)
    return c


def _host_prep(inp, kk=K, aa=A):
    x = np.float32(inp["x"])
    c = _coeffs(inp, kk, aa)                             # (kk, OUT, IN)
    c0sum_v = c[0].sum(axis=1) + np.float32(inp["bias"])  # (OUT,) DC + bias
    c[0] = np.float32(inp["base_weight"])                # k=0 slot: base path
    maps = []
    for g in range(NG):
        isl = slice(g * IG, (g + 1) * IG)
        # cw[p, k, oc, m] = c[k, oc*128+m, g*128+p]
        cw = np.ascontiguousarray(
            c[:, :, isl].transpose(2, 0, 1).reshape(IG, kk, NOC, 128)
        ).astype(ml_dtypes.bfloat16)
        for h in range(NH):
            bsl = slice(h * BH, (h + 1) * BH)
            xT = np.ascontiguousarray(x[bsl, isl].T)     # (IG, BH)
            maps.append({"xT": xT, "cw": cw, "biasv": None, "g": g})
    # The cos-series k=0 (DC) slot was replaced by the base path, so its
    # contribution sum_i c0[o,i] plus the bias is added via the biasv
    # vector - on the g==0 cores only (zeros elsewhere to avoid double add).
    for m in maps:
        if m["g"] == 0:
            m["biasv"] = np.ascontiguousarray(
                c0sum_v.reshape(NOC, 128).T.astype(np.float32))  # (128, NOC)
        else:
            m["biasv"] = np.zeros((128, NOC), np.float32)
        del m["g"]
    return maps


def kernel(**inputs):
    global _nc_cache, LAST_RESULT
    np_in = {k: np.asarray(v, dtype=np.float32) for k, v in inputs.items()}
    if _nc_cache is None:
        _nc_cache = _build_nc()
    in_maps = _host_prep(np_in)
    res = run_bass_kernel_spmd(
        _nc_cache, in_maps, core_ids=list(range(NCORES)), trace=TRACE)
    LAST_RESULT = res
    # results[c]: partial (NOC, 128, BH) for core c = (g, h)
    full = np.zeros((B, OUT), np.float32)
    for ci, r in enumerate(res.results):
        g, h = divmod(ci, NH)
        part = np.asarray(r["out"], np.float32).reshape(OUT, BH)
        full[h * BH:(h + 1) * BH, :] += part.T
    return full



# revision 6
# speedup vs baseline: 2.9361x; 2.9361x over previous
"""ChirpletKANLinear forward on 8 Trainium2 NeuronCores.

Math (per reference):
    base_out[b,o]  = sum_i silu(x[b,i]) * BW[o,i]
    xs             = (x[b,i] - T[o,i]) / S[o,i]
    out[b,o]       = base_out + sum_i cos(2pi*F*xs)*exp(-0.5*xs^2)*CW[o,i]
                     + bias[o]

Algorithm: each chirplet atom h_oi(x) = CW*cos(2pi*F*(x-T)/S)*
exp(-0.5*((x-T)/S)^2) is a fixed smooth function of the scalar x on
x in [-a, a].  Expand it in a shared Fourier-cosine basis
    h_oi(x) ~= sum_{k<K} c[k,o,i] * cos(k*theta),
    theta = pi*(clip(x,-a,a)+a)/(2a)
with coefficients c (DCT-II of h_oi, host-precomputed from the weight
tensors only).  The layer becomes ONE dense matmul with contraction
(i,k) plus K cheap feature tiles depending on x alone.  K=26, a=3.4
gives rel err ~6e-3 in bf16 vs the 2e-2 budget.

Feature generation (the work-balance trick):
  * ODD k (13 of them) use the proven int-phase-wrap chain:
      DVE : mf = i32(round(2^18*(k*xc/(4a) + (k+1)/4)))   (per k)
      DVE : w  = mf << 14       (wraps phase mod 2^32 = mod 2pi,
                                 ONE grouped op per 5 k's)
      ACT : f_k = Sin(w * 2pi/2^32) = cos(k*theta) -> bf16
            (ONE grouped op per 5 k's - scale/bias identical for all k,
             amortizing the ~352-cycle ACT fixed cost)
  * EVEN k are DVE products of odd features (1 bf16 tensor_tensor each):
      k=2m (m odd): F_k = f_m^2        cos(k) = 2F_k - 1
      k=4n:         F_k = f_a*f_b      (a,b = k/2-+1, both odd)
                                       cos(k) = 2F_k - cos(2theta)
    The x2 / -1 / -cos(2theta) terms fold into the host-packed
    coefficients (slot 2) and a per-core bias vector - zero extra
    device work.
  * PE: psum[oc] += c[k,oc-chunk]^T @ F_k   (4 chunks of 128 outs)

Steady-state per-core engine load (per iteration): PE ~22us (104 MMs,
the roofline for this contraction), DVE ~18us, ACT ~7us.

Sharding: 8 cores = 4 in_feature groups (128 each) x 2 batch halves
(512 each).  Every core computes a partial (512 out, 512 batch) fp32
product over its 128 in-features; the host sums the 4 partials per
batch half (the unshard step).
"""

import math

import numpy as np
import ml_dtypes

import concourse.bass as bass
import concourse.bacc as bacc
import concourse.tile as tile
import concourse.mybir as mybir
from concourse.bass_utils import run_bass_kernel_spmd

B, IN, OUT = 1024, 512, 512
NCORES = 8
NG = 4                      # in-feature groups
NH = 2                      # batch halves
IG = IN // NG               # 128 in-features per group
BH = B // NH                # 512 batch per half
NOC = OUT // 128            # 4 output chunks of 128

A = 3.4                     # cosine-series half-range
K = 26                      # coeff slots: k=0 silu, 1..K-1 cos
NQ = 512                    # DCT quadrature points (host)

ODDS = [k for k in range(1, K) if k % 2 == 1]          # 13 odd features
ODD_GROUPS = [ODDS[0:5], ODDS[5:10], ODDS[10:]]        # 5,5,3
EVENS = [k for k in range(2, K) if k % 2 == 0]         # 12 even features
# even k -> (a, b) odd factor pair: F_k = f_a * f_b
EVEN_FACT = {k: ((k // 2, k // 2) if (k // 2) % 2 == 1
                 else (k // 2 - 1, k // 2 + 1)) for k in EVENS}
# device slot order for the coefficient tensor
SLOTS = [0] + ODDS + EVENS
SLOT_OF = {k: i for i, k in enumerate(SLOTS)}

F32 = mybir.dt.float32
I32 = mybir.dt.int32
BF16 = mybir.dt.bfloat16
AF = mybir.ActivationFunctionType
ALU = mybir.AluOpType
TWO_PI = 2.0 * math.pi

TRACE = False
LAST_RESULT = None

_nc_cache = None


def _build_nc(loop_r=None, kk=K, aa=A):
    assert kk == K, "feature plan is precomputed for K"
    nc = bacc.Bacc("TRN2", target_bir_lowering=False, debug=False,
                   num_devices=NCORES)

    xT_d = nc.dram_tensor("xT", [IG, BH], F32, kind="ExternalInput")
    cw_d = nc.dram_tensor("cw", [IG, kk, NOC, 128], BF16,
                          kind="ExternalInput")
    bias_d = nc.dram_tensor("biasv", [128, NOC], F32, kind="ExternalInput")
    out_d = nc.dram_tensor("out", [NOC, 128, BH], F32, kind="ExternalOutput")

    with tile.TileContext(nc) as tc:
        with (
            tc.tile_pool(name="singles", bufs=1) as singles,
            tc.tile_pool(name="mfpool", bufs=2) as mfpool,
            tc.tile_pool(name="wpool", bufs=2) as wpool,
            tc.tile_pool(name="ofpool", bufs=3) as ofpool,
            tc.tile_pool(name="efpool", bufs=4) as efpool,
            tc.tile_pool(name="psum", bufs=1,
                         space=bass.MemorySpace.PSUM) as psump,
        ):
            xT_sb = singles.tile([IG, BH], F32)
            nc.sync.dma_start(xT_sb[:], xT_d[:])
            bias_sb = singles.tile([128, NOC], F32)
            nc.sync.dma_start(bias_sb[:], bias_d[:])
            cw_sb = singles.tile([IG, kk, NOC, 128], BF16)
            nc.sync.dma_start(cw_sb[:], cw_d[:])
            # clamp to [-A, A] so the basis never sees out-of-range x
            xc_sb = singles.tile([IG, BH], F32)
            nc.vector.tensor_scalar(xc_sb, xT_sb, -aa, aa, ALU.max, ALU.min)

            psum_acc = psump.tile([128, NOC, BH], F32)

            def compute_body():
                # base path: silu(x) against base_weight
                f0 = efpool.tile([IG, BH], BF16, tag="f0")
                nc.scalar.activation(f0, xT_sb, AF.Silu)
                for oc in range(NOC):
                    nc.tensor.matmul(
                        psum_acc[:, oc, :], cw_sb[:, 0, oc, :], f0,
                        start=True, stop=False, skip_group_check=True)

                # odd features: per-k affine, grouped shift, grouped Sin
                ofs = {}
                for gi, og in enumerate(ODD_GROUPS):
                    gsz = len(og)
                    mf_g = mfpool.tile([IG, gsz, BH], I32, tag=f"mf{gi}")
                    for j, k in enumerate(og):
                        nc.vector.tensor_scalar(
                            mf_g[:, j, :], xc_sb,
                            float(2 ** 18) * k / (4 * aa),
                            float(2 ** 16) * (k + 1),
                            ALU.mult, ALU.add)
                    w_g = wpool.tile([IG, gsz, BH], I32, tag=f"w{gi}")
                    nc.vector.tensor_scalar(w_g, mf_g, 14, 0,
                                            ALU.arith_shift_left,
                                            ALU.arith_shift_right)
                    f_g = ofpool.tile([IG, gsz, BH], BF16, tag=f"of{gi}")
                    nc.scalar.activation(f_g, w_g, AF.Sin, bias=0.0,
                                         scale=TWO_PI / 2 ** 32)
                    for j, k in enumerate(og):
                        ofs[k] = f_g[:, j, :]
                        for oc in range(NOC):
                            nc.tensor.matmul(
                                psum_acc[:, oc, :],
                                cw_sb[:, SLOT_OF[k], oc, :], f_g[:, j, :],
                                start=False, stop=False,
                                skip_group_check=True)

                # even features: one bf16 TT product each
                for ei, k in enumerate(EVENS):
                    a_, b_ = EVEN_FACT[k]
                    fe = efpool.tile([IG, BH], BF16, tag="fe")
                    nc.vector.tensor_tensor(fe, ofs[a_], ofs[b_], ALU.mult)
                    last = ei == len(EVENS) - 1
                    for oc in range(NOC):
                        nc.tensor.matmul(
                            psum_acc[:, oc, :],
                            cw_sb[:, SLOT_OF[k], oc, :], fe,
                            start=False, stop=last and oc == NOC - 1,
                            skip_group_check=True)

            if loop_r:
                with tc.For_i(0, loop_r, 1,
                              hint_engines=(mybir.EngineType.Activation,
                                            mybir.EngineType.DVE,
                                            mybir.EngineType.PE)):
                    compute_body()
            else:
                compute_body()

            out_sb = singles.tile([128, NOC, BH], F32)
            for oc in range(NOC):
                nc.scalar.activation(out_sb[:, oc, :], psum_acc[:, oc, :],
                                     AF.Identity,
                                     bias=bias_sb[:, oc:oc + 1], scale=1.0)
                nc.sync.dma_start(out_d[oc], out_sb[:, oc, :])

    nc.compile()
    return nc


def _coeffs(inp, kk=K, aa=A):
    """DCT-II cosine coefficients c[k, o, i] of the chirplet atoms."""
    f = np.float32(inp["frequency"])
    s = np.float32(inp["scale"])
    t = np.float32(inp["translation"])
    cwt = np.float32(inp["chirplet_weights"])
    thq = ((np.arange(NQ) + 0.5) * (math.pi / NQ)).astype(np.float32)
    xq = (2 * aa / math.pi) * thq - aa                     # (NQ,)
    basis = np.cos(np.outer(thq, np.arange(kk))).astype(np.float32)
    basis *= 2.0 / NQ
    basis[:, 0] *= 0.5
    c = np.empty((kk, OUT, IN), np.float32)
    for o0 in range(0, OUT, 64):
        o1 = o0 + 64
        u = (xq[None, None, :] - t[o0:o1, :, None]) / s[o0:o1, :, None]
        h = (np.cos(np.float32(TWO_PI) * f[o0:o1, :, None] * u)
             * np.exp(np.float32(-0.5) * u * u) * cwt[o0:o1, :, None])
        c[:, o0:o1, :] = np.einsum("oiq,qk->koi", h, basis, optimize=True)
    return c


def _host_prep(inp, kk=K, aa=A):
    """Pack device coefficient slots + per-core bias with the product folds.

    Device features: slot0 = silu(x); odd k: cos(k*theta); even k = 2m
    (m odd): F_k = f_m^2; even k = 4n: F_k = f_a*f_b (a,b odd).  Algebra:
      cos(2m*th) = 2*F - 1
      cos(4n*th) = 2*F - cos(2*th)
    so the 4n slots fold -c_k into the cos(2th) coefficient, and every
    "- const" lands in the per-core bias vector (per-core because each
    core only sums its own 128-wide i-slice).
    """
    x = np.float32(inp["x"])
    c = _coeffs(inp, kk, aa)
    c0sum = c[0].sum(axis=1) + np.float32(inp["bias"])   # (OUT,)
    c[0] = np.float32(inp["base_weight"])                # silu slot

    ctot2 = c[2].copy()                  # effective cos(2*theta) coefficient
    for k in EVENS:
        if k != 2 and (k // 2) % 2 == 0:
            ctot2 -= c[k]
    cdev = np.zeros_like(c)
    cdev[SLOT_OF[0]] = c[0]
    for k in ODDS:
        cdev[SLOT_OF[k]] = c[k]
    for k in EVENS:
        cdev[SLOT_OF[k]] = 2 * (ctot2 if k == 2 else c[k])
    # per-(o,i) density of the constant terms; per-core bias = slice-sum
    biasden = -ctot2
    for k in EVENS:
        if k != 2 and (k // 2) % 2 == 1:
            biasden -= c[k]

    maps = []
    for g in range(NG):
        isl = slice(g * IG, (g + 1) * IG)
        cw = np.ascontiguousarray(
            cdev[:, :, isl].transpose(2, 0, 1).reshape(IG, kk, NOC, 128)
        ).astype(ml_dtypes.bfloat16)
        bv = biasden[:, isl].sum(axis=1)
        if g == 0:
            bv = bv + c0sum              # series DC + module bias, once
        biasv = np.ascontiguousarray(
            bv.reshape(NOC, 128).T.astype(np.float32))   # (128, NOC)
        for h in range(NH):
            bsl = slice(h * BH, (h + 1) * BH)
            xT = np.ascontiguousarray(x[bsl, isl].T)     # (IG, BH)
            maps.append({"xT": xT, "cw": cw, "biasv": biasv})
    return maps


def kernel(**inputs):
    global _nc_cache, LAST_RESULT
    np_in = {k: np.asarray(v, dtype=np.float32) for k, v in inputs.items()}
    if _nc_cache is None:
        _nc_cache = _build_nc()
    in_maps = _host_prep(np_in)
    res = run_bass_kernel_spmd(
        _nc_cache, in_maps, core_ids=list(range(NCORES)), trace=TRACE)
    LAST_RESULT = res
    # results[c]: partial (NOC, 128, BH) for core c = (g, h)
    full = np.zeros((B, OUT), np.float32)
    for ci, r in enumerate(res.results):
        g, h = divmod(ci, NH)
        part = np.asarray(r["out"], np.float32).reshape(OUT, BH)
        full[h * BH:(h + 1) * BH, :] += part.T
    return full
